# revision 1
# baseline (speedup 1.0000x reference)
"""Trainium2 Bass kernel for nn_CrossFeatureTransformer.

Shapes (hardcoded): B=16, H=64, N=128, C=256, HID=1024, C8=32.
Sharding: data-parallel over B across 8 cores -> 128 (b,h) pairs per core.

Math restructuring (host-side weight folding, all exact algebra):
  attn_in = k - q + pos;  h1 = relu(attn_in @ sc_w1 + sc_b1)
    = relu(key @ (Wk@sc_w1) + pos_h @ (pos_w2@sc_w1) + beta[bh])
  where beta[bh] = sc_b1 + pos_b2@sc_w1 - LN(query)[bh] @ (Wq@sc_w1)   (host)
  scores = h1 @ sc_w2 + sc_b2 + pen   (pen = -10000 on masked -> exp==0)
  softmax over N per channel; agg = sum_n w * (v + pos_h@pos_w2)  (pos_b2
  dropped there: sum_n w = 1 so it folds into bo' = pos_b2@Wo + bo)
  LN2 affine folded into ff_w1/ff_b1.

Device layout: transposed [C, M] tiles, M = bh*N columns, chunks of 4 bh = 512.
key is transposed to [C, M] bf16 via DMA-transpose (XBAR) on load.
"""

import numpy as np
import ml_dtypes

import concourse.bass as bass
import concourse.bacc as bacc
import concourse.mybir as mybir
from concourse.tile import TileContext
from concourse.bass_utils import run_bass_kernel_spmd

BF16 = mybir.dt.bfloat16
F32 = mybir.dt.float32
F32R = mybir.dt.float32r
AX = mybir.AxisListType
ALU = mybir.AluOpType
ACTF = mybir.ActivationFunctionType

B, H, N, C = 16, 64, 128, 256
HID = 1024
C8 = 32
EPS = 1e-6
NCORES = 8
BH = (B // NCORES) * H          # 128 (b,h) pairs per core
M = BH * N                      # 16384 columns per core
CHUNK_BH = 4
CHUNK = CHUNK_BH * N            # 512
NCHUNK = M // CHUNK             # 32
PEN = -10000.0

_cache = {}


def _build_nc():
    nc = bacc.Bacc("TRN2", target_bir_lowering=False, debug=False)

    # ---- DRAM I/O ----
    keyb = nc.dram_tensor("keyb", [BH, N, C], BF16, kind="ExternalInput")
    query = nc.dram_tensor("query", [BH, C], F32, kind="ExternalInput")
    rposT = nc.dram_tensor("rposT", [4, M], BF16, kind="ExternalInput")
    bcb = nc.dram_tensor("bcb", [C8, M], BF16, kind="ExternalInput")
    pen = nc.dram_tensor("pen", [1, M], BF16, kind="ExternalInput")
    wv = nc.dram_tensor("wv", [C, C], BF16, kind="ExternalInput")
    a1 = nc.dram_tensor("a1", [C, C8], BF16, kind="ExternalInput")
    b1i = nc.dram_tensor("b1i", [2 * C8, C8], BF16, kind="ExternalInput")
    sc2e = nc.dram_tensor("sc2e", [C8 + 1, C], BF16, kind="ExternalInput")
    posw1 = nc.dram_tensor("posw1", [4, C8], BF16, kind="ExternalInput")
    posw2 = nc.dram_tensor("posw2", [C8, C], BF16, kind="ExternalInput")
    posb1 = nc.dram_tensor("posb1", [C8, 1], F32, kind="ExternalInput")
    scb2 = nc.dram_tensor("scb2", [128, 2], F32, kind="ExternalInput")
    wo = nc.dram_tensor("wo", [C, C], F32, kind="ExternalInput")
    bor = nc.dram_tensor("bor", [1, C], F32, kind="ExternalInput")
    ff1 = nc.dram_tensor("ff1", [C, HID], BF16, kind="ExternalInput")
    ff2 = nc.dram_tensor("ff2", [HID, C], BF16, kind="ExternalInput")
    ffb1 = nc.dram_tensor("ffb1", [128, HID // 128], F32, kind="ExternalInput")
    ffb2 = nc.dram_tensor("ffb2", [1, C], F32, kind="ExternalInput")
    out = nc.dram_tensor("out", [BH, C], F32, kind="ExternalOutput")

    with TileContext(nc) as tc, tc.tile_pool(name="consts", bufs=1) as cpool:
        def T(shape, dtype, name):
            return cpool.tile(shape, dtype, tag=name, name=name)
        # ---- persistent SBUF constants ----
        wv_sb = T([128, 2, C], BF16, "wv_sb")
        nc.sync.dma_start(wv_sb, wv.rearrange("(kt p) n -> p kt n", p=128))
        a1_sb = T([128, 2, C8], BF16, "a1_sb")
        nc.sync.dma_start(a1_sb, a1.rearrange("(kt p) n -> p kt n", p=128))
        b1i_sb = T([128, C8], BF16, "b1i_sb")
        nc.sync.dma_start(b1i_sb[0:2 * C8], b1i[:])
        sc2e_sb = T([128, C], BF16, "sc2e_sb")
        nc.sync.dma_start(sc2e_sb[0:C8 + 1], sc2e[:])
        posw1_sb = T([128, C8], BF16, "posw1_sb")
        nc.sync.dma_start(posw1_sb[0:4], posw1[:])
        posw2_sb = T([128, C], BF16, "posw2_sb")
        nc.sync.dma_start(posw2_sb[0:C8], posw2[:])
        posb1_sb = T([128, 1], F32, "posb1_sb")
        nc.sync.dma_start(posb1_sb[0:C8], posb1[:])
        scb2_sb = T([128, 2], F32, "scb2_sb")
        nc.sync.dma_start(scb2_sb, scb2[:])
        wo_sb = T([128, 2, C], F32, "wo_sb")
        nc.sync.dma_start(wo_sb, wo.rearrange("(kt p) n -> p kt n", p=128))
        bor_sb = T([128, C], F32, "bor_sb")
        nc.sync.dma_start(bor_sb[0:1], bor[:])
        ff1_sb = T([128, 2, HID], BF16, "ff1_sb")
        nc.sync.dma_start(ff1_sb, ff1.rearrange("(kt p) n -> p kt n", p=128))
        ff2_sb = T([128, 8, C], BF16, "ff2_sb")
        nc.sync.dma_start(ff2_sb, ff2.rearrange("(kt p) n -> p kt n", p=128))
        ffb1_sb = T([128, HID // 128], F32, "ffb1_sb")
        nc.sync.dma_start(ffb1_sb, ffb1[:])
        ffb2_sb = T([128, C], F32, "ffb2_sb")
        nc.sync.dma_start(ffb2_sb[0:1], ffb2[:])
        query_sb = T([BH, C], F32, "query_sb")
        nc.sync.dma_start(query_sb, query[:])

        ones1_sb = T([128, 128], F32, "ones1_sb")
        nc.vector.memset(ones1_sb[0:1], 1.0)
        ident_sb = T([128, 128], F32, "ident_sb")
        from concourse.masks import make_identity
        make_identity(nc, ident_sb)
        ident16_sb = T([128, 128], BF16, "ident16_sb")
        nc.vector.tensor_copy(ident16_sb, ident_sb)

        if True:
            with (
                tc.tile_pool(name="io", bufs=3) as io_pool,
                tc.tile_pool(name="work", bufs=2) as work_pool,
                tc.tile_pool(name="ps_small", bufs=1, space="PSUM") as ps_small,
                tc.tile_pool(name="ps_big", bufs=2, space="PSUM") as ps_big,
            ):
                agg_sb = T([128, 2, BH], F32, "agg_sb")

                for i in range(NCHUNK):
                    cs = slice(i * CHUNK, (i + 1) * CHUNK)
                    b0 = i * CHUNK_BH

                    # keyT chunk via DMA-transpose (XBAR), bf16
                    ktile = io_pool.tile([128, 2, CHUNK], BF16, tag="kt",
                                         name="ktile")
                    for ct in range(2):
                        nc.sync.dma_start_transpose(
                            ktile[:, ct, :],
                            keyb[b0:b0 + CHUNK_BH, :, ct * 128:(ct + 1) * 128]
                            .rearrange("b n c -> (b n) c"))

                    # pos MLP layer 1 -> pos_h [32, CHUNK]; posb rows 0:32
                    # posb rows 32:64 = beta broadcast (DMA)
                    posb = io_pool.tile([128, CHUNK], BF16, tag="posb", name="posb")
                    nc.sync.dma_start(posb[C8:2 * C8, :], bcb[:, cs])
                    rpt = io_pool.tile([128, CHUNK], BF16, tag="rpt", name="rpt")
                    nc.sync.dma_start(rpt[0:4], rposT[:, cs])
                    ph_ps = ps_small.tile([C8, CHUNK], F32, tag="ph", name="ph_ps")
                    nc.tensor.matmul(ph_ps, posw1_sb[0:4], rpt[0:4], start=True, stop=True)
                    nc.scalar.activation(posb[0:C8, :], ph_ps, ACTF.Relu,
                                         bias=posb1_sb[0:C8], scale=1.0)

                    # h1 = relu(A1.T@keyT + [B1;I].T@[pos_h;betabc]) -> [32, CHUNK]
                    h1_ps = ps_small.tile([C8, CHUNK], F32, tag="h1", name="h1_ps")
                    nc.tensor.matmul(h1_ps, a1_sb[:, 0, :], ktile[:, 0, :],
                                     start=True, stop=False)
                    nc.tensor.matmul(h1_ps, a1_sb[:, 1, :], ktile[:, 1, :],
                                     start=False, stop=False)
                    nc.tensor.matmul(h1_ps, b1i_sb[0:2 * C8], posb[0:2 * C8], start=False, stop=True)
                    h1e = io_pool.tile([128, CHUNK], BF16, tag="h1e", name="h1e")
                    nc.sync.dma_start(h1e[C8:C8 + 1, :], pen[:, cs])
                    nc.scalar.activation(h1e[0:C8, :], h1_ps, ACTF.Relu,
                                         bias=0.0, scale=1.0)

                    # scores + vp per c-tile; e = exp(scores); prod = e * vp
                    ep = work_pool.tile([128, 2, 2, CHUNK], BF16, tag="ep", name="ep")
                    dn = work_pool.tile([128, 2, 2, CHUNK_BH], F32, tag="dn", name="dn")
                    for ct in range(2):
                        csl = slice(ct * 128, (ct + 1) * 128)
                        sc_ps = ps_big.tile([128, CHUNK], F32, tag="sc", name="sc_ps")
                        nc.tensor.matmul(sc_ps, sc2e_sb[0:C8 + 1, csl], h1e[0:C8 + 1],
                                         start=True, stop=True)
                        nc.scalar.activation(ep[:, ct, 0, :], sc_ps, ACTF.Exp,
                                             bias=scb2_sb[:, ct:ct + 1], scale=1.0)

                        vp_ps = ps_big.tile([128, CHUNK], F32, tag="vp", name="vp_ps")
                        nc.tensor.matmul(vp_ps, wv_sb[:, 0, csl], ktile[:, 0, :],
                                         start=True, stop=False)
                        nc.tensor.matmul(vp_ps, wv_sb[:, 1, csl], ktile[:, 1, :],
                                         start=False, stop=False)
                        nc.tensor.matmul(vp_ps, posw2_sb[0:C8, csl], posb[0:C8, :],
                                         start=False, stop=True)
                        # prod = e * vp  (DVE, vp read from PSUM)
                        nc.vector.tensor_tensor(ep[:, ct, 1, :], ep[:, ct, 0, :],
                                                vp_ps, ALU.mult)
                        # [d; num] = sum over n (innermost 128) of [e; prod]
                        nc.vector.tensor_reduce(
                            dn[:, ct], ep[:, ct].rearrange("p t (b n) -> p t b n", n=N),
                            axis=AX.X, op=ALU.add)

                    # agg[:, :, 4bh] = num / d
                    rec = work_pool.tile([128, 2, CHUNK_BH], F32, tag="rec", name="rec")
                    nc.vector.reciprocal(rec, dn[:, :, 0, :])
                    nc.vector.tensor_tensor(
                        agg_sb[:, :, i * CHUNK_BH:(i + 1) * CHUNK_BH],
                        dn[:, :, 1, :], rec, ALU.mult)

                # ---- tail: attn_out, residual, LN2, FF, output ----
                at_ps = ps_big.tile([BH, C], F32, tag="sc", name="at_ps")
                nc.tensor.matmul(at_ps, agg_sb[:, 0, :],
                                 wo_sb[:, 0, :], start=True, stop=False)
                nc.tensor.matmul(at_ps, agg_sb[:, 1, :],
                                 wo_sb[:, 1, :], start=False, stop=False)
                nc.tensor.matmul(at_ps, ones1_sb[0:1], bor_sb[0:1],
                                 start=False, stop=True)
                x2_sb = T([BH, C], F32, "x2_sb")
                nc.vector.tensor_tensor(x2_sb, at_ps, query_sb, ALU.add)

                # LN2 (affine folded into ff_w1/ff_b1 on host)
                scol = T([BH, 1], F32, "scol")
                nc.vector.tensor_reduce(scol, x2_sb, axis=AX.X, op=ALU.add)
                mcol = T([BH, 1], F32, "mcol")
                nc.vector.tensor_scalar_mul(mcol, scol, 1.0 / C)
                xc_sb = T([BH, C], F32, "xc_sb")
                nc.vector.tensor_scalar(xc_sb, x2_sb, mcol, None, op0=ALU.subtract)
                sq_sb = T([BH, C], F32, "sq_sb")
                ss_col = T([BH, 1], F32, "ss_col")
                nc.scalar.activation(sq_sb, xc_sb, ACTF.Square, accum_out=ss_col)
                std_col = T([BH, 1], F32, "std_col")
                eps_col = T([BH, 1], F32, "eps_col")
                nc.vector.memset(eps_col, EPS)
                nc.scalar.activation(std_col, ss_col, ACTF.Sqrt,
                                     bias=eps_col, scale=1.0 / C)
                rstd_col = T([BH, 1], F32, "rstd_col")
                nc.vector.reciprocal(rstd_col, std_col)
                y0_sb = T([BH, C], F32, "y0_sb")
                nc.vector.tensor_scalar(y0_sb, xc_sb, rstd_col, None, op0=ALU.mult)

                # y0T (bf16) via PE transpose
                y0t_sb = T([128, 2, BH], BF16, "y0t_sb")
                for ct in range(2):
                    tp_ps = ps_small.tile([128, 128], F32, tag="h1", name="tp_ps")
                    nc.tensor.transpose(tp_ps,
                                        y0_sb[:, ct * 128:(ct + 1) * 128],
                                        ident_sb)
                    nc.vector.tensor_copy(y0t_sb[:, ct, :], tp_ps)

                # FF
                ht_sb = T([128, 8, BH], BF16, "ht_sb")
                for ht in range(8):
                    hsl = slice(ht * 128, (ht + 1) * 128)
                    ff_ps = ps_big.tile([128, BH], F32, tag="vp", name="ff_ps")
                    nc.tensor.matmul(ff_ps, ff1_sb[:, 0, hsl], y0t_sb[:, 0, :],
                                     start=True, stop=False)
                    nc.tensor.matmul(ff_ps, ff1_sb[:, 1, hsl], y0t_sb[:, 1, :],
                                     start=False, stop=True)
                    nc.scalar.activation(ht_sb[:, ht, :], ff_ps, ACTF.Relu,
                                         bias=ffb1_sb[:, ht:ht + 1], scale=1.0)
                y_ps = ps_big.tile([BH, C], F32, tag="sc", name="y_ps")
                for ht in range(8):
                    nc.tensor.matmul(y_ps, ht_sb[:, ht, :], ff2_sb[:, ht, :],
                                     start=(ht == 0), stop=False)
                nc.tensor.matmul(y_ps, ones1_sb[0:1, 0:BH],
                                 ffb2_sb[0:1], start=False, stop=True)
                out_sb = T([BH, C], F32, "out_sb")
                nc.vector.tensor_tensor(out_sb, y_ps, x2_sb, ALU.add)
                nc.sync.dma_start(out[:], out_sb)

    nc.compile()
    return nc


def _ln_np(x, g, b):
    m = x.mean(-1, keepdims=True)
    v = ((x - m) ** 2).mean(-1, keepdims=True)
    return (x - m) / np.sqrt(v + EPS) * g + b


def _prep(inputs):
    f = {k: np.asarray(v, np.float64) for k, v in inputs.items()
         if k != "visibility_mask"}
    mask = np.asarray(inputs["visibility_mask"])

    A1 = f["Wk"] @ f["sc_w1"]                       # [C, 32]
    B1 = f["pos_w2"] @ f["sc_w1"]                   # [32, 32]
    c1 = f["pos_b2"] @ f["sc_w1"] + f["sc_b1"]      # [32]
    q = _ln_np(f["query_input"], f["ln1_g"], f["ln1_b"]) @ f["Wq"]  # [B,H,C]
    beta = c1[None, None] - q @ f["sc_w1"]          # [B,H,32]
    b1i = np.vstack([B1, np.eye(C8)])               # [64, 32]
    sc2e = np.vstack([f["sc_w2"], np.ones((1, C))])  # [33, C]
    bo2 = f["pos_b2"] @ f["Wo"] + f["bo"]           # [C]
    ff1 = np.diag(f["ln2_g"]) @ f["ff_w1"]          # [C, HID]
    ffb1 = f["ln2_b"] @ f["ff_w1"] + f["ff_b1"]     # [HID]

    bf = ml_dtypes.bfloat16
    shared = {
        "wv": f["Wv"].astype(bf),
        "a1": A1.astype(bf),
        "b1i": b1i.astype(bf),
        "sc2e": sc2e.astype(bf),
        "posw1": f["pos_w1"].astype(bf),
        "posw2": f["pos_w2"].astype(bf),
        "posb1": f["pos_b1"].reshape(C8, 1).astype(np.float32),
        "scb2": f["sc_b2"].reshape(2, 128).T.copy().astype(np.float32),
        "wo": f["Wo"].astype(np.float32),
        "bor": bo2.reshape(1, C).astype(np.float32),
        "ff1": ff1.astype(bf),
        "ff2": f["ff_w2"].astype(bf),
        "ffb1": ffb1.reshape(8, 128).T.copy().astype(np.float32),
        "ffb2": f["ff_b2"].reshape(1, C).astype(np.float32),
    }

    key = np.asarray(inputs["key_input"])           # [B,H,N,C] f32
    quer = np.asarray(inputs["query_input"])        # [B,H,C] f32
    rpos = np.asarray(inputs["relative_pos"])       # [B,H,N,4] f32
    penv = np.where(mask[..., 0] == 0, PEN, 0.0)    # [B,H,N]

    in_maps = []
    bpc = B // NCORES
    for c in range(NCORES):
        bs = slice(c * bpc, (c + 1) * bpc)
        m_ = {}
        m_["keyb"] = key[bs].reshape(BH, N, C).astype(bf)
        m_["query"] = quer[bs].reshape(BH, C).astype(np.float32)
        m_["rposT"] = np.ascontiguousarray(
            rpos[bs].reshape(M, 4).T).astype(bf)
        m_["bcb"] = np.ascontiguousarray(
            np.repeat(beta[bs].reshape(BH, C8), N, axis=0).T).astype(bf)
        m_["pen"] = penv[bs].reshape(1, M).astype(bf)
        m_.update(shared)
        in_maps.append(m_)
    return in_maps


def kernel(**inputs):
    if "nc" not in _cache:
        _cache["nc"] = _build_nc()
    nc = _cache["nc"]
    in_maps = _prep(inputs)
    res = run_bass_kernel_spmd(nc, in_maps, core_ids=list(range(NCORES)))
    outs = [r["out"].reshape(B // NCORES, H, C) for r in res.results]
    return np.concatenate(outs, axis=0).astype(np.float32)



# revision 5
# speedup vs baseline: 1.9963x; 1.9963x over previous
"""Trainium2 Bass kernel for nn_CrossFeatureTransformer (V2, transposed-out).

Same folding as V1 (see kernel.py docstring), but scores/vp are produced in
[n, c] orientation per (b,h) pair: the per-bh slice of the [c8, m]-layout h1e
tile doubles as the transposed stationary operand, so no transposes are
needed. The softmax n-reductions (den = sum_n e, num = sum_n e*vp) then
become tensor-engine contractions over partitions: one N=512 matmul per bh
whose lhsT is an all-ones column at position 4i+b (a sliding slice of a
[128,255] band constant) accumulates [den | num] into psum row 4i+b of a
single persistent bank. Normalization happens once at the tail. The Vector
engine only does the e*vp product per chunk.

All matmuls stay in (128,128) tile mode (zero-padded weights, memset-once
finite padding rows). dennum matmuls for chunk i-1 are issued after sc of
chunk i so the PE never waits on exp/mult.
"""

import numpy as np
import ml_dtypes

import concourse.bass as bass
import concourse.bacc as bacc
import concourse.mybir as mybir
from concourse.tile import TileContext
from concourse.bass_utils import run_bass_kernel_spmd

BF16 = mybir.dt.bfloat16
F32 = mybir.dt.float32
AX = mybir.AxisListType
ALU = mybir.AluOpType
ACTF = mybir.ActivationFunctionType

B, H, N, C = 16, 64, 128, 256
HID = 1024
C8 = 32
EPS = 1e-6
NCORES = 8
BH = (B // NCORES) * H          # 128 (b,h) pairs per core
M = BH * N                      # 16384 columns per core
CHUNK_BH = 4
CHUNK = CHUNK_BH * N            # 512
NCHUNK = M // CHUNK             # 32
PEN = -10000.0

_cache = {}


def _build_nc():
    nc = bacc.Bacc("TRN2", target_bir_lowering=False, debug=False)

    # ---- DRAM I/O ----
    ktT = nc.dram_tensor("ktT", [128, 2, M], BF16, kind="ExternalInput")
    auxd = nc.dram_tensor("auxd", [66, M], BF16, kind="ExternalInput")
    query = nc.dram_tensor("query", [BH, C], F32, kind="ExternalInput")
    a1p = nc.dram_tensor("a1p", [128, 2, 128], BF16, kind="ExternalInput")
    b1ip = nc.dram_tensor("b1ip", [128, 128], BF16, kind="ExternalInput")
    sc2eT = nc.dram_tensor("sc2eT", [128, C], BF16, kind="ExternalInput")
    wvT = nc.dram_tensor("wvT", [128, 2, C], BF16, kind="ExternalInput")
    posw2T = nc.dram_tensor("posw2T", [128, C], BF16, kind="ExternalInput")
    wo = nc.dram_tensor("wo", [128, 2, C], F32, kind="ExternalInput")
    bor = nc.dram_tensor("bor", [1, C], F32, kind="ExternalInput")
    ff1 = nc.dram_tensor("ff1", [128, 2, HID], BF16, kind="ExternalInput")
    ff2 = nc.dram_tensor("ff2", [128, 8, C], BF16, kind="ExternalInput")
    ffb1p = nc.dram_tensor("ffb1p", [1, 8, 128], BF16, kind="ExternalInput")
    ffb2 = nc.dram_tensor("ffb2", [1, C], F32, kind="ExternalInput")
    out = nc.dram_tensor("out", [BH, C], F32, kind="ExternalOutput")

    NKT, NAUX, NH1E, NEP = 4, 4, 3, 3

    with TileContext(nc) as tc, tc.tile_pool(name="consts", bufs=1) as cpool:
        def T(shape, dtype, name):
            return cpool.tile(shape, dtype, tag=name, name=name)

        # ---- persistent SBUF constants ----
        a1p_sb = T([128, 2, 128], BF16, "a1p_sb")
        nc.sync.dma_start(a1p_sb, a1p[:])
        b1ip_sb = T([128, 128], BF16, "b1ip_sb")
        nc.sync.dma_start(b1ip_sb, b1ip[:])
        sc2eT_sb = T([128, C], BF16, "sc2eT_sb")
        nc.sync.dma_start(sc2eT_sb, sc2eT[:])
        wvT_sb = T([128, 2, C], BF16, "wvT_sb")
        nc.sync.dma_start(wvT_sb, wvT[:])
        posw2T_sb = T([128, C], BF16, "posw2T_sb")
        nc.sync.dma_start(posw2T_sb, posw2T[:])
        wo_sb = T([128, 2, C], F32, "wo_sb")
        nc.scalar.dma_start(wo_sb, wo[:])
        ff1_sb = T([128, 2, HID], BF16, "ff1_sb")
        nc.scalar.dma_start(ff1_sb, ff1[:])
        ff2_sb = T([128, 8, C], BF16, "ff2_sb")
        nc.scalar.dma_start(ff2_sb, ff2[:])
        query_sb = T([BH, C], F32, "query_sb")
        nc.scalar.dma_start(query_sb, query[:])

        # bias rows padded to full-K matmuls: row 0 = data, rows 1:128 = 0
        onespad_sb = T([128, 128], F32, "onespad_sb")
        nc.vector.memset(onespad_sb, 0.0)
        nc.vector.memset(onespad_sb[0:1], 1.0)
        borpad_sb = T([128, C], F32, "borpad_sb")
        nc.vector.memset(borpad_sb, 0.0)
        nc.scalar.dma_start(borpad_sb[0:1], bor[:])
        ffb2pad_sb = T([128, C], F32, "ffb2pad_sb")
        nc.vector.memset(ffb2pad_sb, 0.0)
        nc.scalar.dma_start(ffb2pad_sb[0:1], ffb2[:])
        ffb1p_sb = T([128, 8, 128], BF16, "ffb1p_sb")
        nc.vector.memset(ffb1p_sb, 0.0)
        nc.scalar.dma_start(ffb1p_sb[0:1], ffb1p[:])
        onescols_sb = T([128, 128], BF16, "onescols_sb")
        nc.vector.memset(onescols_sb, 0.0)
        nc.vector.memset(onescols_sb[0:1], 1.0)

        # band[:, 127] = 1, else 0; slice [127-j : 255-j] = ones column j
        band_sb = T([128, 255], BF16, "band_sb")
        nc.vector.memset(band_sb, 0.0)
        nc.vector.memset(band_sb[:, 127:128], 1.0)

        ident_sb = T([128, 128], F32, "ident_sb")
        from concourse.masks import make_identity
        make_identity(nc, ident_sb)

        with (
            tc.tile_pool(name="ktp", bufs=NKT) as ktp,
            tc.tile_pool(name="auxp", bufs=NAUX) as auxp,
            tc.tile_pool(name="h1ep", bufs=NH1E) as h1ep,
            tc.tile_pool(name="epp", bufs=NEP) as epp,
            tc.tile_pool(name="ps_h1", bufs=1, space="PSUM") as ps_h1,
            tc.tile_pool(name="ps_sc", bufs=1, space="PSUM") as ps_sc,
            tc.tile_pool(name="ps_vp", bufs=2, space="PSUM") as ps_vp,
            tc.tile_pool(name="ps_dn", bufs=1, space="PSUM") as ps_dn,
        ):
            # persistent [den | num] accumulator rows, one bank
            dnps = ps_dn.tile([128, 2, C], F32, tag="dn", name="dnps")
            prev = None

            def dennum(state):
                ep_p, i_p = state
                for b in range(CHUNK_BH):
                    j = i_p * CHUNK_BH + b
                    nc.tensor.matmul(dnps, band_sb[:, 127 - j:255 - j],
                                     ep_p[:, b, :, :],
                                     start=(j == 0), stop=(j == BH - 1))

            for i in range(NCHUNK):
                cs = slice(i * CHUNK, (i + 1) * CHUNK)

                kt = ktp.tile([128, 2, CHUNK], BF16, tag="kt", name="kt")
                nc.sync.dma_start(kt, ktT[:, :, cs])
                aux = auxp.tile([128, CHUNK], BF16, tag="aux", name="aux")
                if i < NAUX:
                    nc.vector.memset(aux[64:128], 0.0)
                nc.scalar.dma_start(aux[0:66], auxd[:, cs])
                h1e = h1ep.tile([128, CHUNK], BF16, tag="h1e", name="h1e")
                if i < NH1E:
                    nc.vector.memset(h1e[32:64], 0.0)
                    nc.vector.memset(h1e[64:128], 0.0)

                # h1 psum: key@A1 + pos_h@B1 + beta; rows 32/33 = (1-m), 1
                h1ps = ps_h1.tile([128, CHUNK], F32, tag="h1", name="h1ps")
                nc.tensor.matmul(h1ps, a1p_sb[:, 0], kt[:, 0], start=True,
                                 stop=False)
                nc.tensor.matmul(h1ps, a1p_sb[:, 1], kt[:, 1], start=False,
                                 stop=False)
                nc.tensor.matmul(h1ps, b1ip_sb, aux, start=False, stop=True)
                nc.scalar.activation(h1e[0:34], h1ps[0:34], ACTF.Relu)

                # vp_b [n, c] = key_b@Wv + pos_h_b@posw2  (per bh)
                vpps = ps_vp.tile([128, CHUNK_BH, C], F32, tag="vp",
                                  name="vpps")
                for b in range(CHUNK_BH):
                    bs = slice(b * N, (b + 1) * N)
                    nc.tensor.matmul(vpps[:, b, :], kt[:, 0, bs],
                                     wvT_sb[:, 0, :], start=True, stop=False)
                    nc.tensor.matmul(vpps[:, b, :], kt[:, 1, bs],
                                     wvT_sb[:, 1, :], start=False,
                                     stop=False)
                    nc.tensor.matmul(vpps[:, b, :], aux[:, bs],
                                     posw2T_sb, start=False, stop=True)

                # scores_b [n, c] = h1e_b.T @ [sc_w2; -1e4; sc_b2]
                scps = ps_sc.tile([128, CHUNK_BH, C], F32, tag="sc",
                                  name="scps")
                for b in range(CHUNK_BH):
                    bs = slice(b * N, (b + 1) * N)
                    nc.tensor.matmul(scps[:, b, :], h1e[:, bs], sc2eT_sb,
                                     start=True, stop=True)

                # den/num matmuls for the previous chunk keep PE busy while
                # this chunk's exp/mult run
                if prev is not None:
                    dennum(prev)

                # e = exp(scores); prod = e*vp
                ep = epp.tile([128, CHUNK_BH, 2, C], BF16, tag="ep",
                              name="ep")
                nc.scalar.activation(ep[:, :, 0, :], scps, ACTF.Exp)
                nc.vector.tensor_tensor(ep[:, :, 1, :], ep[:, :, 0, :],
                                        vpps, ALU.mult)
                prev = (ep, i)

            dennum(prev)

            # ---- tail: normalize, transpose agg, attn_out, LN2, FF ----
            rec_sb = T([BH, C], F32, "rec_sb")
            nc.vector.reciprocal(rec_sb, dnps[:, 0, :])
            agg2_sb = T([BH, C], F32, "agg2_sb")
            nc.vector.tensor_tensor(agg2_sb, dnps[:, 1, :], rec_sb, ALU.mult)

            aggT_sb = T([128, 2, BH], F32, "aggT_sb")
            for ct in range(2):
                tp_ps = ps_h1.tile([128, 128], F32, tag="h1", name="tp_ps")
                nc.tensor.transpose(tp_ps,
                                    agg2_sb[:, ct * 128:(ct + 1) * 128],
                                    ident_sb)
                nc.vector.tensor_copy(aggT_sb[:, ct, :], tp_ps)

            at_ps = ps_sc.tile([BH, C], F32, tag="sc", name="at_ps")
            nc.tensor.matmul(at_ps, aggT_sb[:, 0, :], wo_sb[:, 0, :],
                             start=True, stop=False)
            nc.tensor.matmul(at_ps, aggT_sb[:, 1, :], wo_sb[:, 1, :],
                             start=False, stop=False)
            nc.tensor.matmul(at_ps, onespad_sb, borpad_sb,
                             start=False, stop=True)
            x2_sb = T([BH, C], F32, "x2_sb")
            nc.vector.tensor_tensor(x2_sb, at_ps, query_sb, ALU.add)

            # LN2 (affine folded into ff_w1/ff_b1 on host)
            scol = T([BH, 1], F32, "scol")
            nc.vector.tensor_reduce(scol, x2_sb, axis=AX.X, op=ALU.add)
            mcol = T([BH, 1], F32, "mcol")
            nc.vector.tensor_scalar_mul(mcol, scol, 1.0 / C)
            xc_sb = T([BH, C], F32, "xc_sb")
            nc.vector.tensor_scalar(xc_sb, x2_sb, mcol, None,
                                    op0=ALU.subtract)
            sq_sb = T([BH, C], F32, "sq_sb")
            ss_col = T([BH, 1], F32, "ss_col")
            nc.scalar.activation(sq_sb, xc_sb, ACTF.Square, accum_out=ss_col)
            std_col = T([BH, 1], F32, "std_col")
            eps_col = T([BH, 1], F32, "eps_col")
            nc.vector.memset(eps_col, EPS)
            nc.scalar.activation(std_col, ss_col, ACTF.Sqrt,
                                 bias=eps_col, scale=1.0 / C)
            rstd_col = T([BH, 1], F32, "rstd_col")
            nc.vector.reciprocal(rstd_col, std_col)
            y0_sb = T([BH, C], F32, "y0_sb")
            nc.vector.tensor_scalar(y0_sb, xc_sb, rstd_col, None,
                                    op0=ALU.mult)

            # y0T (bf16) via PE transpose
            y0t_sb = T([128, 2, BH], BF16, "y0t_sb")
            for ct in range(2):
                tp_ps = ps_h1.tile([128, 128], F32, tag="h1", name="tp_ps")
                nc.tensor.transpose(tp_ps,
                                    y0_sb[:, ct * 128:(ct + 1) * 128],
                                    ident_sb)
                nc.vector.tensor_copy(y0t_sb[:, ct, :], tp_ps)

            # FF: hidden = relu(y0@ff1 + ffb1), out = hidden@ff2 + ffb2
            ht_sb = T([128, 8, BH], BF16, "ht_sb")
            for hw in range(2):
                ff_ps = ps_vp.tile([128, 4, BH], F32, tag="vp", name="ff_ps")
                for hq in range(4):
                    ht = hw * 4 + hq
                    hsl = slice(ht * 128, (ht + 1) * 128)
                    nc.tensor.matmul(ff_ps[:, hq, :], ff1_sb[:, 0, hsl],
                                     y0t_sb[:, 0, :], start=True, stop=False)
                    nc.tensor.matmul(ff_ps[:, hq, :], ff1_sb[:, 1, hsl],
                                     y0t_sb[:, 1, :], start=False,
                                     stop=False)
                    nc.tensor.matmul(ff_ps[:, hq, :], ffb1p_sb[:, ht, :],
                                     onescols_sb, start=False, stop=True)
                nc.scalar.activation(
                    ht_sb[:, hw * 4:(hw + 1) * 4, :], ff_ps, ACTF.Relu)
            y_ps = ps_sc.tile([BH, C], F32, tag="sc", name="y_ps")
            for ht in range(8):
                nc.tensor.matmul(y_ps, ht_sb[:, ht, :], ff2_sb[:, ht, :],
                                 start=(ht == 0), stop=False)
            nc.tensor.matmul(y_ps, onespad_sb, ffb2pad_sb,
                             start=False, stop=True)
            out_sb = T([BH, C], F32, "out_sb")
            nc.vector.tensor_tensor(out_sb, y_ps, x2_sb, ALU.add)
            nc.sync.dma_start(out[:], out_sb)

    nc.compile()
    return nc


def _ln_np(x, g, b):
    m = x.mean(-1, keepdims=True)
    v = ((x - m) ** 2).mean(-1, keepdims=True)
    return (x - m) / np.sqrt(v + EPS) * g + b


def _prep(inputs):
    f = {k: np.asarray(v, np.float64) for k, v in inputs.items()
         if k != "visibility_mask"}
    mask = np.asarray(inputs["visibility_mask"])
    bf = ml_dtypes.bfloat16

    A1 = f["Wk"] @ f["sc_w1"]                       # [C, 32]
    B1 = f["pos_w2"] @ f["sc_w1"]                   # [32, 32]
    c1 = f["pos_b2"] @ f["sc_w1"] + f["sc_b1"]      # [32]
    q = _ln_np(f["query_input"], f["ln1_g"], f["ln1_b"]) @ f["Wq"]  # [B,H,C]
    beta = (c1[None, None] - q @ f["sc_w1"]).astype(np.float32)  # [B,H,32]
    bo2 = f["pos_b2"] @ f["Wo"] + f["bo"]           # [C]
    ff1f = np.diag(f["ln2_g"]) @ f["ff_w1"]         # [C, HID]
    ffb1 = f["ln2_b"] @ f["ff_w1"] + f["ff_b1"]     # [HID]

    a1p = np.zeros((256, 128), np.float64)
    a1p[:, 0:C8] = A1
    a1p = np.ascontiguousarray(
        a1p.reshape(2, 128, 128).transpose(1, 0, 2)).astype(bf)

    b1ip = np.zeros((128, 128), np.float64)
    b1ip[0:C8, 0:C8] = B1
    b1ip[C8:2 * C8, 0:C8] = np.eye(C8)
    b1ip[64, 32] = 1.0                               # (1-m) passthrough row
    b1ip[65, 33] = 1.0                               # const-1 row
    b1ip = b1ip.astype(bf)

    sc2eT = np.zeros((128, C), np.float64)
    sc2eT[0:C8] = f["sc_w2"]
    sc2eT[32] = PEN
    sc2eT[33] = f["sc_b2"]
    sc2eT = sc2eT.astype(bf)

    wvT = np.ascontiguousarray(
        f["Wv"].reshape(2, 128, C).transpose(1, 0, 2)).astype(bf)

    posw2T = np.zeros((128, C), np.float64)
    posw2T[0:C8] = f["pos_w2"]
    posw2T = posw2T.astype(bf)

    shared = {
        "a1p": a1p, "b1ip": b1ip, "sc2eT": sc2eT, "wvT": wvT,
        "posw2T": posw2T,
        "wo": np.ascontiguousarray(
            f["Wo"].reshape(2, 128, C).transpose(1, 0, 2)).astype(np.float32),
        "bor": bo2.reshape(1, C).astype(np.float32),
        "ff1": np.ascontiguousarray(
            ff1f.reshape(2, 128, HID).transpose(1, 0, 2)).astype(bf),
        "ff2": np.ascontiguousarray(
            f["ff_w2"].reshape(8, 128, C).transpose(1, 0, 2)).astype(bf),
        "ffb1p": ffb1.reshape(1, 8, 128).astype(bf),
        "ffb2": f["ff_b2"].reshape(1, C).astype(np.float32),
    }

    key = np.asarray(inputs["key_input"], np.float32)    # [B,H,N,C]
    quer = np.asarray(inputs["query_input"], np.float32)  # [B,H,C]
    rpos = np.asarray(inputs["relative_pos"], np.float32)  # [B,H,N,4]
    pos_h = np.maximum(
        rpos @ f["pos_w1"].astype(np.float32)
        + f["pos_b1"].astype(np.float32), 0.0)           # [B,H,N,32]
    inv_mask = (mask[..., 0] == 0).astype(np.float32)    # [B,H,N]

    in_maps = []
    bpc = B // NCORES
    for c in range(NCORES):
        bs = slice(c * bpc, (c + 1) * bpc)
        m_ = {}
        ktc = key[bs].reshape(M, C).T                    # [C, M]
        m_["ktT"] = np.ascontiguousarray(
            ktc.reshape(2, 128, M).transpose(1, 0, 2)).astype(bf)
        aux = np.empty((66, M), np.float32)
        aux[0:32] = pos_h[bs].reshape(M, C8).T
        aux[32:64] = np.repeat(beta[bs].reshape(BH, C8), N, axis=0).T
        aux[64] = inv_mask[bs].reshape(M)
        aux[65] = 1.0
        m_["auxd"] = aux.astype(bf)
        m_["query"] = quer[bs].reshape(BH, C).astype(np.float32)
        m_.update(shared)
        in_maps.append(m_)
    return in_maps


def kernel(**inputs):
    if "nc" not in _cache:
        _cache["nc"] = _build_nc()
    nc = _cache["nc"]
    in_maps = _prep(inputs)
    res = run_bass_kernel_spmd(nc, in_maps, core_ids=list(range(NCORES)))
    outs = [r["out"].reshape(B // NCORES, H, C) for r in res.results]
    return np.concatenate(outs, axis=0).astype(np.float32)


# revision 12
# speedup vs baseline: 2.3200x; 1.1621x over previous
"""Trainium2 Bass kernel for nn_CrossFeatureTransformer (V2, transposed-out).

Same folding as V1 (see kernel.py docstring), but scores/vp are produced in
[n, c] orientation per (b,h) pair: the per-bh slice of the [c8, m]-layout h1e
tile doubles as the transposed stationary operand, so no transposes are
needed. The softmax n-reductions (den = sum_n e, num = sum_n e*vp) then
become tensor-engine contractions over partitions: one N=512 matmul per bh
whose lhsT is an all-ones column at position 4i+b (a sliding slice of a
[128,255] band constant) accumulates [den | num] into psum row 4i+b of a
single persistent bank. Normalization happens once at the tail. The Vector
engine only does the e*vp product per chunk.

All matmuls stay in (128,128) tile mode (zero-padded weights, memset-once
finite padding rows). dennum matmuls for chunk i-1 are issued after sc of
chunk i so the PE never waits on exp/mult.
"""

import numpy as np
import ml_dtypes

import concourse.bass as bass
import concourse.bacc as bacc
import concourse.mybir as mybir
from concourse.tile import TileContext
from concourse.bass_utils import run_bass_kernel_spmd

BF16 = mybir.dt.bfloat16
FP8 = mybir.dt.float8e4
F32 = mybir.dt.float32
DR = mybir.MatmulPerfMode.DoubleRow
AX = mybir.AxisListType
ALU = mybir.AluOpType
ACTF = mybir.ActivationFunctionType

B, H, N, C = 16, 64, 128, 256
HID = 1024
C8 = 32
EPS = 1e-6
NCORES = 8
BH = (B // NCORES) * H          # 128 (b,h) pairs per core
M = BH * N                      # 16384 columns per core
CHUNK_BH = 4
CHUNK = CHUNK_BH * N            # 512
NCHUNK = M // CHUNK             # 32
PEN = -10000.0

_cache = {}


def _build_nc():
    nc = bacc.Bacc("TRN2", target_bir_lowering=False, debug=False)

    # ---- DRAM I/O ----
    ktT = nc.dram_tensor("ktT", [128, 2, M], FP8, kind="ExternalInput")
    auxd = nc.dram_tensor("auxd", [66, M], BF16, kind="ExternalInput")
    query = nc.dram_tensor("query", [BH, C], F32, kind="ExternalInput")
    a1p = nc.dram_tensor("a1p", [128, 2, 128], FP8, kind="ExternalInput")
    b1ip = nc.dram_tensor("b1ip", [128, 128], BF16, kind="ExternalInput")
    sc2eT = nc.dram_tensor("sc2eT", [128, C], BF16, kind="ExternalInput")
    wvT = nc.dram_tensor("wvT", [128, 2, C], FP8, kind="ExternalInput")
    posw2T = nc.dram_tensor("posw2T", [128, C], BF16, kind="ExternalInput")
    wo = nc.dram_tensor("wo", [128, 2, C], F32, kind="ExternalInput")
    bor = nc.dram_tensor("bor", [1, C], F32, kind="ExternalInput")
    ff1 = nc.dram_tensor("ff1", [128, 2, HID], BF16, kind="ExternalInput")
    ff2 = nc.dram_tensor("ff2", [128, 8, C], BF16, kind="ExternalInput")
    ffb1p = nc.dram_tensor("ffb1p", [1, 8, 128], BF16, kind="ExternalInput")
    ffb2 = nc.dram_tensor("ffb2", [1, C], F32, kind="ExternalInput")
    out = nc.dram_tensor("out", [BH, C], F32, kind="ExternalOutput")

    NKT, NAUX, NH1E, NEP = 6, 6, 4, 4

    with TileContext(nc) as tc, tc.tile_pool(name="consts", bufs=1) as cpool:
        def T(shape, dtype, name):
            return cpool.tile(shape, dtype, tag=name, name=name)

        # ---- persistent SBUF constants ----
        a1p_sb = T([128, 2, 128], FP8, "a1p_sb")
        nc.sync.dma_start(a1p_sb, a1p[:])
        b1ip_sb = T([128, 128], BF16, "b1ip_sb")
        nc.sync.dma_start(b1ip_sb, b1ip[:])
        sc2eT_sb = T([128, C], BF16, "sc2eT_sb")
        nc.sync.dma_start(sc2eT_sb, sc2eT[:])
        wvT_sb = T([128, 2, C], FP8, "wvT_sb")
        nc.sync.dma_start(wvT_sb, wvT[:])
        posw2T_sb = T([128, C], BF16, "posw2T_sb")
        nc.sync.dma_start(posw2T_sb, posw2T[:])
        # tail-only weights: DMA'd from inside the loop (i==1) so they don't
        # delay the first chunks' kt/aux transfers
        wo_sb = T([128, 2, C], F32, "wo_sb")
        ff1_sb = T([128, 2, HID], BF16, "ff1_sb")
        ff2_sb = T([128, 8, C], BF16, "ff2_sb")
        query_sb = T([BH, C], F32, "query_sb")

        # bias rows padded to full-K matmuls: row 0 = data, rows 1:128 = 0
        onespad_sb = T([128, 128], F32, "onespad_sb")
        nc.vector.memset(onespad_sb, 0.0)
        nc.vector.memset(onespad_sb[0:1], 1.0)
        borpad_sb = T([128, C], F32, "borpad_sb")
        nc.vector.memset(borpad_sb, 0.0)
        nc.scalar.dma_start(borpad_sb[0:1], bor[:])
        ffb2pad_sb = T([128, C], F32, "ffb2pad_sb")
        nc.vector.memset(ffb2pad_sb, 0.0)
        nc.scalar.dma_start(ffb2pad_sb[0:1], ffb2[:])
        ffb1p_sb = T([128, 8, 128], BF16, "ffb1p_sb")
        nc.vector.memset(ffb1p_sb, 0.0)
        nc.scalar.dma_start(ffb1p_sb[0:1], ffb1p[:])
        onescols_sb = T([128, 128], BF16, "onescols_sb")
        nc.vector.memset(onescols_sb, 0.0)
        nc.vector.memset(onescols_sb[0:1], 1.0)

        # band[:, 127] = 1, else 0; slice [127-j : 255-j] = ones column j
        band_sb = T([128, 255], BF16, "band_sb")
        nc.vector.memset(band_sb, 0.0)
        nc.vector.memset(band_sb[:, 127:128], 1.0)

        ident_sb = T([128, 128], F32, "ident_sb")
        from concourse.masks import make_identity
        make_identity(nc, ident_sb)

        with (
            tc.tile_pool(name="ktp", bufs=NKT) as ktp,
            tc.tile_pool(name="auxp", bufs=NAUX) as auxp,
            tc.tile_pool(name="h1ep", bufs=NH1E) as h1ep,
            tc.tile_pool(name="epp", bufs=NEP) as epp,
            tc.tile_pool(name="ps_h1", bufs=1, space="PSUM") as ps_h1,
            tc.tile_pool(name="ps_sc", bufs=1, space="PSUM") as ps_sc,
            tc.tile_pool(name="ps_vp", bufs=2, space="PSUM") as ps_vp,
            tc.tile_pool(name="ps_dn", bufs=1, space="PSUM") as ps_dn,
        ):
            # persistent [den | num] accumulator rows, one bank
            dnps = ps_dn.tile([128, 2, C], F32, tag="dn", name="dnps")
            prev = None

            def dennum(state):
                ep_p, i_p = state
                for b in range(CHUNK_BH):
                    j = i_p * CHUNK_BH + b
                    nc.tensor.matmul(dnps, band_sb[:, 127 - j:255 - j],
                                     ep_p[:, b, :, :],
                                     start=(j == 0), stop=(j == BH - 1))

            for i in range(NCHUNK):
                cs = slice(i * CHUNK, (i + 1) * CHUNK)

                kt = ktp.tile([128, 2, CHUNK], FP8, tag="kt", name="kt")
                nc.sync.dma_start(kt, ktT[:, :, cs])
                aux = auxp.tile([128, CHUNK], BF16, tag="aux", name="aux")
                if i < NAUX:
                    nc.vector.memset(aux[64:128], 0.0)
                nc.sync.dma_start(aux[0:66], auxd[:, cs])
                h1e = h1ep.tile([128, CHUNK], BF16, tag="h1e", name="h1e")
                if i < NH1E:
                    nc.vector.memset(h1e[32:64], 0.0)
                    nc.vector.memset(h1e[64:128], 0.0)
                if i == 1:
                    nc.scalar.dma_start(wo_sb, wo[:])
                    nc.scalar.dma_start(ff1_sb, ff1[:])
                    nc.scalar.dma_start(ff2_sb, ff2[:])
                    nc.scalar.dma_start(query_sb, query[:])

                # h1 psum: key@A1 + pos_h@B1 + beta; rows 32/33 = (1-m), 1
                h1ps = ps_h1.tile([128, CHUNK], F32, tag="h1", name="h1ps")
                nc.tensor.matmul(h1ps, a1p_sb, kt, start=True,
                                 stop=False, perf_mode=DR)
                nc.tensor.matmul(h1ps, b1ip_sb, aux, start=False, stop=True)
                nc.scalar.activation(h1e[0:34], h1ps[0:34], ACTF.Relu)

                # vp_b [n, c] = key_b@Wv + pos_h_b@posw2  (per bh)
                vpps = ps_vp.tile([128, CHUNK_BH, C], F32, tag="vp",
                                  name="vpps")
                for b in range(CHUNK_BH):
                    bs = slice(b * N, (b + 1) * N)
                    nc.tensor.matmul(vpps[:, b, :], kt[:, :, bs],
                                     wvT_sb, start=True, stop=False,
                                     perf_mode=DR)
                    nc.tensor.matmul(vpps[:, b, :], aux[:, bs],
                                     posw2T_sb, start=False, stop=True)

                # scores_b [n, c] = h1e_b.T @ [sc_w2; -1e4; sc_b2]
                scps = ps_sc.tile([128, CHUNK_BH, C], F32, tag="sc",
                                  name="scps")
                for b in range(CHUNK_BH):
                    bs = slice(b * N, (b + 1) * N)
                    nc.tensor.matmul(scps[:, b, :], h1e[:, bs], sc2eT_sb,
                                     start=True, stop=True)

                # den/num matmuls for the previous chunk keep PE busy while
                # this chunk's exp/mult run
                if prev is not None:
                    dennum(prev)

                # e = exp(scores); prod = e*vp
                ep = epp.tile([128, CHUNK_BH, 2, C], BF16, tag="ep",
                              name="ep")
                nc.scalar.activation(ep[:, :, 0, :], scps, ACTF.Exp)
                nc.vector.tensor_tensor(ep[:, :, 1, :], ep[:, :, 0, :],
                                        vpps, ALU.mult)
                prev = (ep, i)

            dennum(prev)

            # ---- tail: normalize, transpose agg, attn_out, LN2, FF ----
            rec_sb = T([BH, C], F32, "rec_sb")
            nc.vector.reciprocal(rec_sb, dnps[:, 0, :])
            agg2_sb = T([BH, C], F32, "agg2_sb")
            nc.vector.tensor_tensor(agg2_sb, dnps[:, 1, :], rec_sb, ALU.mult)

            aggT_sb = T([128, 2, BH], F32, "aggT_sb")
            for ct in range(2):
                tp_ps = ps_h1.tile([128, 128], F32, tag="h1", name="tp_ps")
                nc.tensor.transpose(tp_ps,
                                    agg2_sb[:, ct * 128:(ct + 1) * 128],
                                    ident_sb)
                nc.vector.tensor_copy(aggT_sb[:, ct, :], tp_ps)

            at_ps = ps_sc.tile([BH, C], F32, tag="sc", name="at_ps")
            nc.tensor.matmul(at_ps, aggT_sb[:, 0, :], wo_sb[:, 0, :],
                             start=True, stop=False)
            nc.tensor.matmul(at_ps, aggT_sb[:, 1, :], wo_sb[:, 1, :],
                             start=False, stop=False)
            nc.tensor.matmul(at_ps, onespad_sb, borpad_sb,
                             start=False, stop=True)
            x2_sb = T([BH, C], F32, "x2_sb")
            nc.vector.tensor_tensor(x2_sb, at_ps, query_sb, ALU.add)

            # LN2 (affine folded into ff_w1/ff_b1 on host)
            scol = T([BH, 1], F32, "scol")
            nc.vector.tensor_reduce(scol, x2_sb, axis=AX.X, op=ALU.add)
            mcol = T([BH, 1], F32, "mcol")
            nc.vector.tensor_scalar_mul(mcol, scol, 1.0 / C)
            xc_sb = T([BH, C], F32, "xc_sb")
            nc.vector.tensor_scalar(xc_sb, x2_sb, mcol, None,
                                    op0=ALU.subtract)
            sq_sb = T([BH, C], F32, "sq_sb")
            ss_col = T([BH, 1], F32, "ss_col")
            nc.scalar.activation(sq_sb, xc_sb, ACTF.Square, accum_out=ss_col)
            std_col = T([BH, 1], F32, "std_col")
            eps_col = T([BH, 1], F32, "eps_col")
            nc.vector.memset(eps_col, EPS)
            nc.scalar.activation(std_col, ss_col, ACTF.Sqrt,
                                 bias=eps_col, scale=1.0 / C)
            rstd_col = T([BH, 1], F32, "rstd_col")
            nc.vector.reciprocal(rstd_col, std_col)
            y0_sb = T([BH, C], F32, "y0_sb")
            nc.vector.tensor_scalar(y0_sb, xc_sb, rstd_col, None,
                                    op0=ALU.mult)

            # y0T (bf16) via PE transpose
            y0t_sb = T([128, 2, BH], BF16, "y0t_sb")
            for ct in range(2):
                tp_ps = ps_h1.tile([128, 128], F32, tag="h1", name="tp_ps")
                nc.tensor.transpose(tp_ps,
                                    y0_sb[:, ct * 128:(ct + 1) * 128],
                                    ident_sb)
                nc.vector.tensor_copy(y0t_sb[:, ct, :], tp_ps)

            # FF: hidden = relu(y0@ff1 + ffb1), out = hidden@ff2 + ffb2
            ht_sb = T([128, 8, BH], BF16, "ht_sb")
            for hw in range(2):
                ff_ps = ps_vp.tile([128, 4, BH], F32, tag="vp", name="ff_ps")
                for hq in range(4):
                    ht = hw * 4 + hq
                    hsl = slice(ht * 128, (ht + 1) * 128)
                    nc.tensor.matmul(ff_ps[:, hq, :], ff1_sb[:, 0, hsl],
                                     y0t_sb[:, 0, :], start=True, stop=False)
                    nc.tensor.matmul(ff_ps[:, hq, :], ff1_sb[:, 1, hsl],
                                     y0t_sb[:, 1, :], start=False,
                                     stop=False)
                    nc.tensor.matmul(ff_ps[:, hq, :], ffb1p_sb[:, ht, :],
                                     onescols_sb, start=False, stop=True)
                nc.scalar.activation(
                    ht_sb[:, hw * 4:(hw + 1) * 4, :], ff_ps, ACTF.Relu)
            y_ps = ps_sc.tile([BH, C], F32, tag="sc", name="y_ps")
            for ht in range(8):
                nc.tensor.matmul(y_ps, ht_sb[:, ht, :], ff2_sb[:, ht, :],
                                 start=(ht == 0), stop=False)
            nc.tensor.matmul(y_ps, onespad_sb, ffb2pad_sb,
                             start=False, stop=True)
            out_sb = T([BH, C], F32, "out_sb")
            nc.vector.tensor_tensor(out_sb, y_ps, x2_sb, ALU.add)
            nc.sync.dma_start(out[:], out_sb)

    nc.compile()
    return nc


def _ln_np(x, g, b):
    m = x.mean(-1, keepdims=True)
    v = ((x - m) ** 2).mean(-1, keepdims=True)
    return (x - m) / np.sqrt(v + EPS) * g + b


def _prep(inputs):
    f = {k: np.asarray(v, np.float64) for k, v in inputs.items()
         if k != "visibility_mask"}
    mask = np.asarray(inputs["visibility_mask"])
    bf = ml_dtypes.bfloat16
    f8 = ml_dtypes.float8_e4m3

    A1 = f["Wk"] @ f["sc_w1"]                       # [C, 32]
    B1 = f["pos_w2"] @ f["sc_w1"]                   # [32, 32]
    c1 = f["pos_b2"] @ f["sc_w1"] + f["sc_b1"]      # [32]
    q = _ln_np(f["query_input"], f["ln1_g"], f["ln1_b"]) @ f["Wq"]  # [B,H,C]
    beta = (c1[None, None] - q @ f["sc_w1"]).astype(np.float32)  # [B,H,32]
    bo2 = f["pos_b2"] @ f["Wo"] + f["bo"]           # [C]
    ff1f = np.diag(f["ln2_g"]) @ f["ff_w1"]         # [C, HID]
    ffb1 = f["ln2_b"] @ f["ff_w1"] + f["ff_b1"]     # [HID]

    a1p = np.zeros((256, 128), np.float64)
    a1p[:, 0:C8] = A1
    a1p = np.ascontiguousarray(
        a1p.reshape(2, 128, 128).transpose(1, 0, 2)).astype(f8)

    b1ip = np.zeros((128, 128), np.float64)
    b1ip[0:C8, 0:C8] = B1
    b1ip[C8:2 * C8, 0:C8] = np.eye(C8)
    b1ip[64, 32] = 1.0                               # (1-m) passthrough row
    b1ip[65, 33] = 1.0                               # const-1 row
    b1ip = b1ip.astype(bf)

    sc2eT = np.zeros((128, C), np.float64)
    sc2eT[0:C8] = f["sc_w2"]
    sc2eT[32] = PEN
    sc2eT[33] = f["sc_b2"]
    sc2eT = sc2eT.astype(bf)

    wvT = np.ascontiguousarray(
        f["Wv"].reshape(2, 128, C).transpose(1, 0, 2)).astype(f8)

    posw2T = np.zeros((128, C), np.float64)
    posw2T[0:C8] = f["pos_w2"]
    posw2T = posw2T.astype(bf)

    shared = {
        "a1p": a1p, "b1ip": b1ip, "sc2eT": sc2eT, "wvT": wvT,
        "posw2T": posw2T,
        "wo": np.ascontiguousarray(
            f["Wo"].reshape(2, 128, C).transpose(1, 0, 2)).astype(np.float32),
        "bor": bo2.reshape(1, C).astype(np.float32),
        "ff1": np.ascontiguousarray(
            ff1f.reshape(2, 128, HID).transpose(1, 0, 2)).astype(bf),
        "ff2": np.ascontiguousarray(
            f["ff_w2"].reshape(8, 128, C).transpose(1, 0, 2)).astype(bf),
        "ffb1p": ffb1.reshape(1, 8, 128).astype(bf),
        "ffb2": f["ff_b2"].reshape(1, C).astype(np.float32),
    }

    key = np.asarray(inputs["key_input"], np.float32)    # [B,H,N,C]
    quer = np.asarray(inputs["query_input"], np.float32)  # [B,H,C]
    rpos = np.asarray(inputs["relative_pos"], np.float32)  # [B,H,N,4]
    pos_h = np.maximum(
        rpos @ f["pos_w1"].astype(np.float32)
        + f["pos_b1"].astype(np.float32), 0.0)           # [B,H,N,32]
    inv_mask = (mask[..., 0] == 0).astype(np.float32)    # [B,H,N]

    in_maps = []
    bpc = B // NCORES
    for c in range(NCORES):
        bs = slice(c * bpc, (c + 1) * bpc)
        m_ = {}
        ktc = key[bs].reshape(M, C).T                    # [C, M]
        m_["ktT"] = np.ascontiguousarray(
            ktc.reshape(2, 128, M).transpose(1, 0, 2)).astype(f8)
        aux = np.empty((66, M), np.float32)
        aux[0:32] = pos_h[bs].reshape(M, C8).T
        aux[32:64] = np.repeat(beta[bs].reshape(BH, C8), N, axis=0).T
        aux[64] = inv_mask[bs].reshape(M)
        aux[65] = 1.0
        m_["auxd"] = aux.astype(bf)
        m_["query"] = quer[bs].reshape(BH, C).astype(np.float32)
        m_.update(shared)
        in_maps.append(m_)
    return in_maps


def kernel(**inputs):
    if "nc" not in _cache:
        _cache["nc"] = _build_nc()
    nc = _cache["nc"]
    in_maps = _prep(inputs)
    res = run_bass_kernel_spmd(nc, in_maps, core_ids=list(range(NCORES)))
    outs = [r["out"].reshape(B // NCORES, H, C) for r in res.results]
    return np.concatenate(outs, axis=0).astype(np.float32)
